# revision 14
# baseline (speedup 1.0000x reference)
"""Trainium2 Bass kernel for nn_AttentionNet (pooling / ridge regime).

Model (per batch b of B=128, L=512, D=300, H=200, V=50000):
  word_emb = emb_table[words]                          [B,L,D]
  subj_emb = max over l with subj_pos[b,l]==0 of word_emb (else -1e12)
  obj_emb  = same with obj_pos
  hid  = tanh(word_emb @ w1[:D] + subj_emb @ w1[D:] + b1)
  attn = softmax(hid @ w2, axis=l)    (b2 dropped: softmax shift-invariant)
  subj_attn = sum_l attn * word_emb   (obj_attn identical -- source bug)
  out = relu(relu(cat([subj_attn, subj_attn, subj_emb, obj_emb]) @ mw1 + mb1) @ mw2 + mb2)

Sharding: pure data parallel, 16 batches per core on 8 cores; embedding
table and the small weights replicated.

All embeddings/weights are bf16 on device; PSUM accumulation, softmax
stats, biases and the final output stay fp32.

Device plan per core (16 batches = 16 token-tiles of 512):
  - bulk gather via gpsimd.dma_gather (int16 indices).  The table is
    stored with one sentinel row (-1e12) at physical row 0 and one at
    row V+1 (word w -> physical row w+1).  The vocabulary exceeds int16
    range, so each batch's tokens are sorted by word id (attention +
    pools are order-invariant within a batch) and split into the 256
    smallest / 256 largest; low halves gather from table[0:32768], high
    halves from table[VP-32768:VP].
  - masked max-pools via a second, transposed gather: for each
    (batch, mask) the host emits the ~64 unmasked token ids padded with
    sentinel rows to 64 lo + 64 hi slots; dma_gather(transpose=True)
    lands them D-major and a single segmented reduce_max per side plus
    a combine/clamp produces subj/obj embeddings with zero mask
    arithmetic on the compute engines.
  - attention scores via bf16 matmuls on D-major PE-transposed
    embeddings, softmax on-chip, attention-weighted sum with the
    attention column as the stationary matmul operand.
  - 2-layer output MLP with the duplicated subj_attn block pre-folded
    into mw1 on the host (rows 0:300 += rows 300:600).
"""

import numpy as np

import concourse.bass as bass
import concourse.bacc as bacc
import concourse.mybir as mybir
import concourse.tile as tile
from concourse.masks import make_identity
from contextlib import ExitStack

F32 = mybir.dt.float32
BF16 = mybir.dt.bfloat16
I16 = mybir.dt.int16

NEG_INF = 1e12      # reference constant

# ---------------------------------------------------------------- config


class Cfg:
    def __init__(self, B=128, L=512, D=300, H=200, V=50000, NCORES=8,
                 PT=128, CW=128, HCW=100, gather_split=2, PSLOT=64):
        self.B, self.L, self.D, self.H, self.V = B, L, D, H, V
        self.NCORES = NCORES
        self.BC = B // NCORES          # batches per core
        self.PT = PT                   # token subtile (partitions)
        self.NSUB = L // PT            # subtiles per batch (must be even)
        self.NS = self.BC * self.NSUB  # token subtiles per core
        self.T = self.BC * L           # tokens per core
        self.CW = CW                   # D-chunk width
        self.HCW = HCW                 # H-chunk width
        self.gather_split = gather_split
        assert L % PT == 0 and H % HCW == 0 and self.NSUB % 2 == 0
        # gather element size in bf16 elements: row bytes padded to 256B
        self.E = -(-D * 2 // 256) * 128          # 384 for D=300
        # sentinel-augmented table: word w -> physical row w + 1
        self.VP = V + 2
        self.HB2 = self.VP - 32768     # high window start (17234)
        # pool slots per (batch, mask) per side
        self.PSLOT = PSLOT
        self.NPOOL = 2 * self.BC * PSLOT   # pool rows per side per core
        # exact chunks of D (last may be narrow)
        self.dch = []
        s = 0
        while s < D:
            self.dch.append((s, min(CW, D - s)))
            s += CW
        self.hch = [(i * HCW, HCW) for i in range(H // HCW)]
        self.nd = len(self.dch)
        self.nh = len(self.hch)
        assert self.nd * CW == self.E  # bf16 rows tile exactly into chunks

    def subtiles(self, b):
        """Global subtile ids of batch b: low half then high half."""
        h = self.NSUB // 2
        lo = [h * b + k for k in range(h)]
        hi = [self.NS // 2 + h * b + k for k in range(h)]
        return lo + hi


# ------------------------------------------------------------- device IR


def build_nc(cfg: Cfg):
    c = cfg
    nc = bacc.Bacc(num_swdge_queues=4)

    NH16 = (c.T // 2) // 16
    NP16 = c.NPOOL // 16
    idxlo_d = nc.declare_dram_parameter("idx_lo", [128, NH16], I16, isOutput=False)
    idxhi_d = nc.declare_dram_parameter("idx_hi", [128, NH16], I16, isOutput=False)
    idxpl_d = nc.declare_dram_parameter("idx_pl", [128, NP16], I16, isOutput=False)
    idxph_d = nc.declare_dram_parameter("idx_ph", [128, NP16], I16, isOutput=False)
    table = nc.declare_dram_parameter("table", [c.VP, c.E], BF16, isOutput=False)
    w1a_d = nc.declare_dram_parameter("w1a", [c.D, c.H], BF16, isOutput=False)
    w1b_d = nc.declare_dram_parameter("w1b", [c.D, c.H], BF16, isOutput=False)
    b1_d = nc.declare_dram_parameter("b1", [c.H, 1], F32, isOutput=False)
    w2_d = nc.declare_dram_parameter("w2", [c.H, 1], BF16, isOutput=False)
    mw1_d = nc.declare_dram_parameter("mw1e", [3 * c.D, c.H], BF16, isOutput=False)
    mb1_d = nc.declare_dram_parameter("mb1", [c.H, 1], F32, isOutput=False)
    mw2_d = nc.declare_dram_parameter("mw2", [c.H, c.H], BF16, isOutput=False)
    mb2_d = nc.declare_dram_parameter("mb2", [c.H, 1], F32, isOutput=False)
    out_d = nc.declare_dram_parameter("out", [c.nh, c.HCW, c.BC], F32, isOutput=True)

    with tile.TileContext(nc) as tc, ExitStack() as ctx:
        sb = ctx.enter_context(tc.tile_pool(name="sb", bufs=1))

        # ---- persistent SBUF tiles
        ixl_sb = sb.tile([128, NH16], I16)
        ixh_sb = sb.tile([128, NH16], I16)
        ixpl_sb = sb.tile([128, NP16], I16)
        ixph_sb = sb.tile([128, NP16], I16)
        emb_tok = sb.tile([c.PT, c.NS, c.E], BF16)
        emb_dT = sb.tile([c.CW, c.NS, c.nd, c.PT], BF16)  # [dlow, subtile, chunk, tok]
        pool_lo = sb.tile([c.PT, c.NPOOL // c.PT, c.E], BF16)
        pool_hi = sb.tile([c.PT, c.NPOOL // c.PT, c.E], BF16)
        pcomb = sb.tile([c.PT, c.NPOOL // c.PT, c.E], BF16)
        pooled = sb.tile([c.CW, c.nd, 2 * c.BC], BF16)   # [dlow, chunk, m*BC+b]
        w1a_sb = sb.tile([c.CW, c.nd, c.H], BF16)
        w1b_sb = sb.tile([c.CW, c.nd, c.H], BF16)
        w2_sb = sb.tile([c.HCW, c.nh], BF16)
        b1_sb = sb.tile([c.HCW, c.nh], F32)
        mw1_sb = sb.tile([c.CW, 3 * c.nd, c.H], BF16)
        mb1_sb = sb.tile([c.HCW, c.nh], F32)
        mw2_sb = sb.tile([c.HCW, c.nh, c.H], BF16)
        mb2_sb = sb.tile([c.HCW, c.nh], F32)
        ident = sb.tile([c.PT, c.PT], BF16)
        bias_sb = sb.tile([c.HCW, c.nh, c.BC], F32)
        scores = sb.tile([c.BC, c.L], F32)
        attn = sb.tile([c.BC, c.L], F32)
        attn_bf = sb.tile([c.BC, c.L], BF16)
        attn_t = sb.tile([c.PT, c.NSUB, c.BC], BF16)
        sattn = sb.tile([c.BC, c.D], F32)
        sattn_bf = sb.tile([c.BC, c.D], BF16)
        satd = sb.tile([c.CW, c.nd, c.BC], BF16)
        smax = sb.tile([c.BC, 1], F32)
        nsmax = sb.tile([c.BC, 1], F32)
        ssum = sb.tile([c.BC, 1], F32)
        srec = sb.tile([c.BC, 1], F32)
        o1_sb = sb.tile([c.HCW, c.nh, c.BC], BF16)
        out_sb = sb.tile([c.HCW, c.nh, c.BC], F32)

        # ---- load indices & weights
        nc.sync.dma_start(out=ixl_sb[:], in_=idxlo_d[:])
        nc.sync.dma_start(out=ixh_sb[:], in_=idxhi_d[:])
        nc.sync.dma_start(out=ixpl_sb[:], in_=idxpl_d[:])
        nc.sync.dma_start(out=ixph_sb[:], in_=idxph_d[:])
        for ci, (d0, dn) in enumerate(c.dch):
            nc.sync.dma_start(out=w1a_sb[0:dn, ci, :], in_=w1a_d[d0:d0 + dn, :])
            nc.sync.dma_start(out=w1b_sb[0:dn, ci, :], in_=w1b_d[d0:d0 + dn, :])
            for blk in range(3):
                nc.sync.dma_start(out=mw1_sb[0:dn, blk * c.nd + ci, :],
                                  in_=mw1_d[blk * c.D + d0:blk * c.D + d0 + dn, :])
        for hi, (h0, hn) in enumerate(c.hch):
            nc.sync.dma_start(out=w2_sb[0:hn, hi:hi + 1], in_=w2_d[h0:h0 + hn, :])
            nc.sync.dma_start(out=b1_sb[0:hn, hi:hi + 1], in_=b1_d[h0:h0 + hn, :])
            nc.sync.dma_start(out=mb1_sb[0:hn, hi:hi + 1], in_=mb1_d[h0:h0 + hn, :])
            nc.sync.dma_start(out=mb2_sb[0:hn, hi:hi + 1], in_=mb2_d[h0:h0 + hn, :])
            nc.sync.dma_start(out=mw2_sb[0:hn, hi, :], in_=mw2_d[h0:h0 + hn, :])
        make_identity(nc, ident[:])

        # ---- pool gathers: token-major [slot%128, slot//128, elem]
        nc.gpsimd.dma_gather(
            out_ap=pool_lo[:], in_ap=table[0:32768, :],
            idxs_ap=ixpl_sb[:], num_idxs=c.NPOOL,
            num_idxs_reg=c.NPOOL, elem_size=c.E,
            single_packet=False, queue_num=0)
        nc.gpsimd.dma_gather(
            out_ap=pool_hi[:], in_ap=table[c.HB2:c.VP, :],
            idxs_ap=ixph_sb[:], num_idxs=c.NPOOL,
            num_idxs_reg=c.NPOOL, elem_size=c.E,
            single_packet=False, queue_num=1)

        # ---- bulk gathers: low halves -> subtiles [0, NS/2), high halves after
        NHALF = c.T // 2
        nsp = c.gather_split
        npc = NHALF // nsp
        assert npc % 128 == 0, (NHALF, nsp)
        for k in range(nsp):
            i0, s0 = k * (npc // 16), k * (npc // 128)
            nc.gpsimd.dma_gather(
                out_ap=emb_tok[:, s0:s0 + npc // 128, :], in_ap=table[0:32768, :],
                idxs_ap=ixl_sb[:, i0:i0 + npc // 16], num_idxs=npc, num_idxs_reg=npc,
                elem_size=c.E, single_packet=False, queue_num=(2 + 2 * k) % 4)
            nc.gpsimd.dma_gather(
                out_ap=emb_tok[:, c.NS // 2 + s0:c.NS // 2 + s0 + npc // 128, :],
                in_ap=table[c.HB2:c.VP, :],
                idxs_ap=ixh_sb[:, i0:i0 + npc // 16], num_idxs=npc, num_idxs_reg=npc,
                elem_size=c.E, single_packet=False, queue_num=(3 + 2 * k) % 4)

        # ---- xbar-transpose gathered embeddings to D-major
        # emb_dT[dlow, s, ci, t] = emb_tok[t, s, ci*128 + dlow]
        for k in range(nsp):
            for h in range(2):
                s0 = h * (c.NS // 2) + k * (npc // 128)
                eng = nc.sync if (2 * k + h) % 2 == 0 else nc.scalar
                eng.dma_start_transpose(
                    out=emb_dT[:, s0:s0 + npc // 128, :, :],
                    in_=emb_tok[:, s0:s0 + npc // 128, :])

        # ---- pools: combine lo/hi slotwise, PE-transpose each slot subtile,
        # segmented reduce over the two 64-slot halves, clamp.
        nc.vector.tensor_tensor(out=pcomb[:], in0=pool_lo[:], in1=pool_hi[:],
                                op=mybir.AluOpType.max)
        with tc.tile_pool(name="ppool", bufs=2, space="PSUM") as ppool:
            for st in range(c.NPOOL // c.PT):
                pp = ppool.tile([c.CW, c.nd, c.PT], BF16, tag="pp")
                for ci in range(c.nd):
                    nc.tensor.transpose(
                        out=pp[:, ci, :],
                        in_=pcomb[:, st, ci * c.CW:(ci + 1) * c.CW],
                        identity=ident[:])
                nc.vector.tensor_reduce(
                    out=pooled[:, :, 2 * st:2 * st + 2],
                    in_=pp[:].rearrange("p c (q s) -> p c q s", s=c.PSLOT),
                    axis=mybir.AxisListType.X, op=mybir.AluOpType.max)
        nc.vector.tensor_scalar_max(out=pooled[:], in0=pooled[:],
                                    scalar1=-NEG_INF)

        def pool_rhs(m, ci, bsl=None):
            dn = c.dch[ci][1]
            if bsl is None:
                bsl = slice(0, c.BC)
            return pooled[0:dn, ci, m * c.BC + bsl.start:m * c.BC + bsl.stop]

        # ---- grouped main loop: group bias -> transposes -> dense hid/scores.
        GB = min(4, c.BC)
        NG = c.BC // GB

        def emb_rhs(b, ci, dn):
            # batch b's 512 tokens: lo subtiles {2b, 2b+1}, hi {NS/2+2b, +1}
            return (emb_dT[0:dn, :, ci, :]
                    .rearrange("p (h s) t -> p h s t", h=2)
                    [:, :, 2 * b:2 * b + 2, :])

        with tc.tile_pool(name="bpool", bufs=1, space="PSUM") as bpool, \
             tc.tile_pool(name="hpool", bufs=4, space="PSUM") as hpool, \
             tc.tile_pool(name="spool", bufs=2, space="PSUM") as spool, \
             tc.tile_pool(name="srpool", bufs=3) as srpool, \
             tc.tile_pool(name="hspool", bufs=2) as hspool:
            for g in range(NG):
                gsl = slice(g * GB, (g + 1) * GB)
                # -- tanh bias for the group: w1b^T subj_emb + b1
                for hi, (h0, hn) in enumerate(c.hch):
                    pb = bpool.tile([c.HCW, GB], F32, tag="pb")
                    for ci, (d0, dn) in enumerate(c.dch):
                        nc.tensor.matmul(
                            out=pb[0:hn, :],
                            lhsT=w1b_sb[0:dn, ci, h0:h0 + hn],
                            rhs=pool_rhs(0, ci, gsl),
                            start=(ci == 0), stop=(ci == c.nd - 1))
                    nc.scalar.activation(
                        out=bias_sb[0:hn, hi, gsl], in_=pb[0:hn, :],
                        func=mybir.ActivationFunctionType.Identity,
                        bias=b1_sb[0:hn, hi:hi + 1], scale=1.0)
                # -- dense hid + scores matmul stream for the group
                for b in range(g * GB, (g + 1) * GB):
                    hid = hspool.tile([c.HCW, c.nh, c.L], BF16, tag="hid")
                    for hi, (h0, hn) in enumerate(c.hch):
                        ph = hpool.tile([c.HCW, c.L], F32, tag="ph")
                        for ci, (d0, dn) in enumerate(c.dch):
                            nc.tensor.matmul(
                                out=ph[0:hn, :],
                                lhsT=w1a_sb[0:dn, ci, h0:h0 + hn],
                                rhs=emb_rhs(b, ci, dn),
                                start=(ci == 0), stop=(ci == c.nd - 1))
                        nc.scalar.activation(
                            out=hid[0:hn, hi, :], in_=ph[0:hn, :],
                            func=mybir.ActivationFunctionType.Tanh,
                            bias=bias_sb[0:hn, hi, b:b + 1], scale=1.0)
                    ps = spool.tile([1, c.L], F32, tag="ps")
                    for hi, (h0, hn) in enumerate(c.hch):
                        nc.tensor.matmul(
                            out=ps[:], lhsT=w2_sb[0:hn, hi:hi + 1],
                            rhs=hid[0:hn, hi, :],
                            start=(hi == 0), stop=(hi == c.nh - 1))
                    srow = srpool.tile([1, c.L], F32, tag="srow")
                    nc.vector.tensor_copy(out=srow[:], in_=ps[:])
                    nc.sync.dma_start(out=scores[b:b + 1, :], in_=srow[:])

        # ---- softmax over L for all batches
        nc.vector.tensor_reduce(out=smax[:], in_=scores[:],
                                axis=mybir.AxisListType.X, op=mybir.AluOpType.max)
        nc.vector.tensor_scalar_mul(out=nsmax[:], in0=smax[:], scalar1=-1.0)
        nc.scalar.activation(out=attn[:], in_=scores[:],
                             func=mybir.ActivationFunctionType.Exp,
                             bias=nsmax[:, 0:1], scale=1.0)
        nc.vector.tensor_reduce(out=ssum[:], in_=attn[:],
                                axis=mybir.AxisListType.X, op=mybir.AluOpType.add)
        nc.vector.reciprocal(out=srec[:], in_=ssum[:])
        nc.vector.tensor_scalar_mul(out=attn[:], in0=attn[:], scalar1=srec[:, 0:1])
        nc.vector.tensor_copy(out=attn_bf[:], in_=attn[:])

        # ---- transpose attn to token-major columns [PT, si, b]
        with tc.tile_pool(name="apool", bufs=2, space="PSUM") as apool:
            for si in range(c.NSUB):
                pa = apool.tile([c.PT, c.BC], BF16, tag="pa")
                nc.tensor.transpose(out=pa[:],
                                    in_=attn_bf[:, si * c.PT:(si + 1) * c.PT],
                                    identity=ident[0:c.BC, 0:c.BC])
                nc.vector.tensor_copy(out=attn_t[:, si, :], in_=pa[:])

        # ---- attention-weighted sum  -> sattn [b, D]
        with tc.tile_pool(name="wpool", bufs=4, space="PSUM") as wpool, \
             tc.tile_pool(name="wrpool", bufs=3) as wrpool:
            for b in range(c.BC):
                subs = c.subtiles(b)
                pw = wpool.tile([1, c.D], F32, tag="pw")
                for si, s in enumerate(subs):
                    nc.tensor.matmul(
                        out=pw[:],
                        lhsT=attn_t[:, si, b:b + 1],
                        rhs=emb_tok[:, s, 0:c.D],
                        start=(si == 0), stop=(si == c.NSUB - 1))
                wrow = wrpool.tile([1, c.D], F32, tag="wrow")
                nc.scalar.copy(out=wrow[:], in_=pw[:])
                nc.sync.dma_start(out=sattn[b:b + 1, :], in_=wrow[:])

        # ---- transpose sattn to D-major chunks [dlow, chunk, b]
        nc.vector.tensor_copy(out=sattn_bf[:], in_=sattn[:])
        with tc.tile_pool(name="stpool", bufs=2, space="PSUM") as stpool:
            for ci, (d0, dn) in enumerate(c.dch):
                pst = stpool.tile([c.CW, c.BC], BF16, tag="pst")
                nc.tensor.transpose(out=pst[0:dn, :], in_=sattn_bf[:, d0:d0 + dn],
                                    identity=ident[0:c.BC, 0:c.BC])
                nc.vector.tensor_copy(out=satd[0:dn, ci, :], in_=pst[0:dn, :])

        # ---- output MLP (N=BC is small)
        with tc.tile_pool(name="mpool", bufs=2, space="PSUM") as mpool, \
             tc.tile_pool(name="m2pool", bufs=2, space="PSUM") as m2pool:
            nk = 3 * c.nd
            for hi, (h0, hn) in enumerate(c.hch):
                pm = mpool.tile([c.HCW, c.BC], F32, tag="pm")
                for blk in range(3):
                    for ci, (d0, dn) in enumerate(c.dch):
                        k = blk * c.nd + ci
                        if blk == 0:
                            rhs = satd[0:dn, ci, :]
                        else:
                            rhs = pool_rhs(blk - 1, ci)
                        nc.tensor.matmul(
                            out=pm[0:hn, :],
                            lhsT=mw1_sb[0:dn, k, h0:h0 + hn],
                            rhs=rhs, start=(k == 0), stop=(k == nk - 1))
                nc.scalar.activation(
                    out=o1_sb[0:hn, hi, :], in_=pm[0:hn, :],
                    func=mybir.ActivationFunctionType.Relu,
                    bias=mb1_sb[0:hn, hi:hi + 1], scale=1.0)
            for hi, (h0, hn) in enumerate(c.hch):
                pm2 = m2pool.tile([c.HCW, c.BC], F32, tag="pm2")
                for ki, (k0, kn) in enumerate(c.hch):
                    nc.tensor.matmul(
                        out=pm2[0:hn, :],
                        lhsT=mw2_sb[0:kn, ki, h0:h0 + hn],
                        rhs=o1_sb[0:kn, ki, :],
                        start=(ki == 0), stop=(ki == c.nh - 1))
                nc.scalar.activation(
                    out=out_sb[0:hn, hi, :], in_=pm2[0:hn, :],
                    func=mybir.ActivationFunctionType.Relu,
                    bias=mb2_sb[0:hn, hi:hi + 1], scale=1.0)
            for hi in range(c.nh):
                nc.sync.dma_start(out=out_d[hi], in_=out_sb[:, hi, :])

    nc.finalize()
    return nc


# ------------------------------------------------------------------ host


def wrap16(idx, n):
    """int16 index list -> [128, n/16] wrapped + replicated per Q7 core."""
    return np.ascontiguousarray(
        np.tile(np.asarray(idx).astype(np.int16).reshape(n // 16, 16).T, (8, 1)))


def to_bf16(x):
    import ml_dtypes
    return np.asarray(x, dtype=np.float32).astype(ml_dtypes.bfloat16)


def host_prepare(cfg: Cfg, words, subj_pos, obj_pos, emb_table,
                 w1, b1, w2, b2, mw1, mb1, mw2, mb2):
    import ml_dtypes
    c = cfg
    words = np.asarray(words).astype(np.int64)
    subj_pos = np.asarray(subj_pos)
    obj_pos = np.asarray(obj_pos)
    f32 = lambda x: np.ascontiguousarray(np.asarray(x, dtype=np.float32))

    # sentinel rows at physical 0 and VP-1; word w -> row w + 1
    table = np.zeros((c.VP, c.E), ml_dtypes.bfloat16)
    table[1:1 + c.V, :c.D] = to_bf16(emb_table)
    table[0, :] = ml_dtypes.bfloat16(-NEG_INF)
    table[c.VP - 1, :] = ml_dtypes.bfloat16(-NEG_INF)

    w1 = np.asarray(w1, dtype=np.float32)
    w1a, w1b = w1[:c.D], w1[c.D:2 * c.D]
    mw1 = np.asarray(mw1, dtype=np.float32)
    mw1e = np.concatenate([mw1[0:c.D] + mw1[c.D:2 * c.D],
                           mw1[2 * c.D:3 * c.D], mw1[3 * c.D:4 * c.D]], axis=0)
    shared = {
        "table": table,
        "w1a": to_bf16(w1a), "w1b": to_bf16(w1b),
        "b1": f32(b1).reshape(c.H, 1),
        "w2": to_bf16(np.asarray(w2).reshape(c.H, 1)),
        "mw1e": to_bf16(mw1e),
        "mb1": f32(mb1).reshape(c.H, 1),
        "mw2": to_bf16(mw2),
        "mb2": f32(mb2).reshape(c.H, 1),
    }
    HALF = c.L // 2
    in_maps = []
    for core in range(c.NCORES):
        b0 = core * c.BC
        lo_list, hi_list = [], []
        # pool slot arrays, ordered q = m*BC + b
        pl = np.zeros((2 * c.BC, c.PSLOT), np.int64)          # sentinel row 0
        ph = np.full((2 * c.BC, c.PSLOT), 32767, np.int64)    # sentinel VP-1
        for b in range(c.BC):
            w = words[b0 + b]
            order = np.argsort(w, kind="stable")
            ws = w[order] + 1                       # physical rows
            if ws[HALF - 1] >= 32768 or ws[HALF] < c.HB2:
                raise RuntimeError(
                    f"batch {b0 + b}: vocab split infeasible "
                    f"(lo_max={ws[HALF - 1]}, hi_min={ws[HALF]})")
            lo_list.append(ws[:HALF])
            hi_list.append(ws[HALF:] - c.HB2)
            for m, pos in ((0, subj_pos), (1, obj_pos)):
                kept = w[np.asarray(pos[b0 + b]) == 0] + 1    # physical rows
                lo = [int(x) for x in kept[kept < c.HB2]]
                hi = [int(x) for x in kept[kept > 32767]]
                for x in kept[(kept >= c.HB2) & (kept <= 32767)]:
                    (lo if len(lo) < c.PSLOT else hi).append(int(x))
                if len(lo) > c.PSLOT or len(hi) > c.PSLOT:
                    raise RuntimeError(f"pool slot overflow b={b0+b} m={m}: "
                                       f"{len(lo)}/{len(hi)}")
                q = m * c.BC + b
                pl[q, :len(lo)] = lo
                ph[q, :len(hi)] = [x - c.HB2 for x in hi]
        in_maps.append({
            "idx_lo": wrap16(np.concatenate(lo_list), c.T // 2),
            "idx_hi": wrap16(np.concatenate(hi_list), c.T // 2),
            "idx_pl": wrap16(pl.reshape(-1), c.NPOOL),
            "idx_ph": wrap16(ph.reshape(-1), c.NPOOL),
            **shared})
    return in_maps


def assemble_output(cfg: Cfg, results):
    c = cfg
    outs = []
    for core in range(c.NCORES):
        o = results[core]["out"]                      # [nh, HCW, BC]
        outs.append(o.reshape(c.H, c.BC).T)           # [BC, H]
    return np.ascontiguousarray(np.concatenate(outs, axis=0))


_CACHE = {}


def run(inputs, trace=False, **kw):
    from concourse.bass_utils import run_bass_kernel_spmd

    cfg = Cfg()
    in_maps = host_prepare(cfg, **{k: inputs[k] for k in (
        "words", "subj_pos", "obj_pos", "emb_table", "w1", "b1", "w2", "b2",
        "mw1", "mb1", "mw2", "mb2")})
    if "nc" not in _CACHE:
        _CACHE["nc"] = build_nc(cfg)
    nc = _CACHE["nc"]
    res = run_bass_kernel_spmd(nc, in_maps, core_ids=list(range(cfg.NCORES)),
                               trace=trace, **kw)
    return assemble_output(cfg, res.results), res


def kernel(**inputs) -> np.ndarray:
    return run(inputs)[0]


# revision 16
# speedup vs baseline: 1.4536x; 1.4536x over previous
"""Trainium2 Bass kernel for nn_AttentionNet (pooling / ridge regime).

Model (per batch b of B=128, L=512, D=300, H=200, V=50000):
  word_emb = emb_table[words]                          [B,L,D]
  subj_emb = max over l with subj_pos[b,l]==0 of word_emb (else -1e12)
  obj_emb  = same with obj_pos
  hid  = tanh(word_emb @ w1[:D] + subj_emb @ w1[D:] + b1)
  attn = softmax(hid @ w2, axis=l)    (b2 dropped: softmax shift-invariant)
  subj_attn = sum_l attn * word_emb   (obj_attn identical -- source bug)
  out = relu(relu(cat([subj_attn, subj_attn, subj_emb, obj_emb]) @ mw1 + mb1) @ mw2 + mb2)

Sharding: pure data parallel, 16 batches per core on 8 cores; embedding
table and the small weights replicated.

All embeddings/weights are bf16 on device; PSUM accumulation, softmax
stats, biases and the final output stay fp32.

Device plan per core (16 batches = 16 token-tiles of 512):
  - bulk gather via gpsimd.dma_gather (int16 indices, 1024 per call --
    larger prep descriptors hit a GPSIMD cost cliff).  The table is
    stored with one sentinel row (-1e12) at physical row 0 and one at
    row V+1 (word w -> physical row w+1).  The vocabulary exceeds int16
    range, so each batch's tokens are sorted by word id (attention +
    pools are order-invariant within a batch) and split into the 256
    smallest / 256 largest; low halves gather from table[0:32768], high
    halves from table[VP-32768:VP].
  - masked max-pools via a second gather: for each (batch, mask) the
    host emits the ~64 unmasked token ids padded with sentinel rows to
    64 lo + 64 hi slots.  Subject slots ship in the first pool calls so
    the tanh bias (w1b^T subj_emb + b1) unblocks before the main GEMM
    needs it; object slots only gate the final MLP.  On device: one
    slotwise lo/hi max, 3 PE transposes per 128-slot subtile, one
    segmented reduce -> both pools with zero mask arithmetic.
  - attention scores via bf16 matmuls on D-major PE-transposed
    embeddings, softmax on-chip, attention-weighted sum with the
    attention column as the stationary matmul operand.
  - 2-layer output MLP with the duplicated subj_attn block pre-folded
    into mw1 on the host (rows 0:300 += rows 300:600).
"""

import numpy as np

import concourse.bass as bass
import concourse.bacc as bacc
import concourse.mybir as mybir
import concourse.tile as tile
from concourse.masks import make_identity
from contextlib import ExitStack

F32 = mybir.dt.float32
BF16 = mybir.dt.bfloat16
I16 = mybir.dt.int16

NEG_INF = 1e12      # reference constant

# ---------------------------------------------------------------- config


class Cfg:
    def __init__(self, B=128, L=512, D=300, H=200, V=50000, NCORES=8,
                 PT=128, CW=128, HCW=100, gather_split=4, PSLOT=64):
        self.B, self.L, self.D, self.H, self.V = B, L, D, H, V
        self.NCORES = NCORES
        self.BC = B // NCORES          # batches per core
        self.PT = PT                   # token subtile (partitions)
        self.NSUB = L // PT            # subtiles per batch (must be even)
        self.NS = self.BC * self.NSUB  # token subtiles per core
        self.T = self.BC * L           # tokens per core
        self.CW = CW                   # D-chunk width
        self.HCW = HCW                 # H-chunk width
        self.gather_split = gather_split
        assert L % PT == 0 and H % HCW == 0 and self.NSUB % 2 == 0
        # gather element size in bf16 elements: row bytes padded to 256B
        self.E = -(-D * 2 // 256) * 128          # 384 for D=300
        # sentinel-augmented table: word w -> physical row w + 1
        self.VP = V + 2
        self.HB2 = self.VP - 32768     # high window start (17234)
        # pool slots per (batch, mask) per side
        self.PSLOT = PSLOT
        self.NPOOL = 2 * self.BC * PSLOT   # pool rows per side per core
        self.NPS = self.NPOOL // self.PT   # pool subtiles per side (16)
        # exact chunks of D (last may be narrow)
        self.dch = []
        s = 0
        while s < D:
            self.dch.append((s, min(CW, D - s)))
            s += CW
        self.hch = [(i * HCW, HCW) for i in range(H // HCW)]
        self.nd = len(self.dch)
        self.nh = len(self.hch)
        assert self.nd * CW == self.E  # bf16 rows tile exactly into chunks

    def subtiles(self, b):
        """Global subtile ids of batch b: low half then high half."""
        h = self.NSUB // 2
        lo = [h * b + k for k in range(h)]
        hi = [self.NS // 2 + h * b + k for k in range(h)]
        return lo + hi


# ------------------------------------------------------------- device IR


def build_nc(cfg: Cfg, queue_map=None):
    c = cfg
    nc = bacc.Bacc(num_swdge_queues=4)

    NH16 = (c.T // 2) // 16
    NP16 = c.NPOOL // 16
    idxlo_d = nc.declare_dram_parameter("idx_lo", [128, NH16], I16, isOutput=False)
    idxhi_d = nc.declare_dram_parameter("idx_hi", [128, NH16], I16, isOutput=False)
    idxpl_d = nc.declare_dram_parameter("idx_pl", [128, NP16], I16, isOutput=False)
    idxph_d = nc.declare_dram_parameter("idx_ph", [128, NP16], I16, isOutput=False)
    table = nc.declare_dram_parameter("table", [c.VP, c.E], BF16, isOutput=False)
    w1a_d = nc.declare_dram_parameter("w1a", [c.D, c.H], BF16, isOutput=False)
    w1b_d = nc.declare_dram_parameter("w1b", [c.D, c.H], BF16, isOutput=False)
    b1_d = nc.declare_dram_parameter("b1", [c.H, 1], F32, isOutput=False)
    w2_d = nc.declare_dram_parameter("w2", [c.H, 1], BF16, isOutput=False)
    mw1_d = nc.declare_dram_parameter("mw1e", [3 * c.D, c.H], BF16, isOutput=False)
    mb1_d = nc.declare_dram_parameter("mb1", [c.H, 1], F32, isOutput=False)
    mw2_d = nc.declare_dram_parameter("mw2", [c.H, c.H], BF16, isOutput=False)
    mb2_d = nc.declare_dram_parameter("mb2", [c.H, 1], F32, isOutput=False)
    out_d = nc.declare_dram_parameter("out", [c.nh, c.HCW, c.BC], F32, isOutput=True)

    with tile.TileContext(nc) as tc, ExitStack() as ctx:
        sb = ctx.enter_context(tc.tile_pool(name="sb", bufs=1))

        # ---- persistent SBUF tiles
        ixl_sb = sb.tile([128, NH16], I16)
        ixh_sb = sb.tile([128, NH16], I16)
        ixpl_sb = sb.tile([128, NP16], I16)
        ixph_sb = sb.tile([128, NP16], I16)
        emb_tok = sb.tile([c.PT, c.NS, c.E], BF16)
        pool_lo = sb.tile([c.PT, c.NPS, c.E], BF16)
        pool_hi = sb.tile([c.PT, c.NPS, c.E], BF16)
        pcomb = sb.tile([c.PT, c.NPS, c.E], BF16)
        pooled = sb.tile([c.CW, c.nd, 2 * c.BC], BF16)   # [dlow, chunk, m*BC+b]
        w1a_sb = sb.tile([c.CW, c.nd, c.H], BF16)
        w1b_sb = sb.tile([c.CW, c.nd, c.H], BF16)
        w2_sb = sb.tile([c.HCW, c.nh], BF16)
        b1_sb = sb.tile([c.HCW, c.nh], F32)
        mw1_sb = sb.tile([c.CW, 3 * c.nd, c.H], BF16)
        mb1_sb = sb.tile([c.HCW, c.nh], F32)
        mw2_sb = sb.tile([c.HCW, c.nh, c.H], BF16)
        mb2_sb = sb.tile([c.HCW, c.nh], F32)
        ident = sb.tile([c.PT, c.PT], BF16)
        bias_sb = sb.tile([c.HCW, c.nh, c.BC], F32)
        scores = sb.tile([c.BC, c.L], F32)
        attn = sb.tile([c.BC, c.L], F32)
        attn_bf = sb.tile([c.BC, c.L], BF16)
        attn_t = sb.tile([c.PT, c.NSUB, c.BC], BF16)
        sattn = sb.tile([c.BC, c.D], F32)
        sattn_bf = sb.tile([c.BC, c.D], BF16)
        satd = sb.tile([c.CW, c.nd, c.BC], BF16)
        smax = sb.tile([c.BC, 1], F32)
        nsmax = sb.tile([c.BC, 1], F32)
        ssum = sb.tile([c.BC, 1], F32)
        srec = sb.tile([c.BC, 1], F32)
        o1_sb = sb.tile([c.HCW, c.nh, c.BC], BF16)
        out_sb = sb.tile([c.HCW, c.nh, c.BC], F32)

        # ---- load indices & weights
        nc.sync.dma_start(out=ixl_sb[:], in_=idxlo_d[:])
        nc.sync.dma_start(out=ixh_sb[:], in_=idxhi_d[:])
        nc.sync.dma_start(out=ixpl_sb[:], in_=idxpl_d[:])
        nc.sync.dma_start(out=ixph_sb[:], in_=idxph_d[:])
        for ci, (d0, dn) in enumerate(c.dch):
            nc.sync.dma_start(out=w1a_sb[0:dn, ci, :], in_=w1a_d[d0:d0 + dn, :])
            nc.sync.dma_start(out=w1b_sb[0:dn, ci, :], in_=w1b_d[d0:d0 + dn, :])
            for blk in range(3):
                nc.sync.dma_start(out=mw1_sb[0:dn, blk * c.nd + ci, :],
                                  in_=mw1_d[blk * c.D + d0:blk * c.D + d0 + dn, :])
        for hi, (h0, hn) in enumerate(c.hch):
            nc.sync.dma_start(out=w2_sb[0:hn, hi:hi + 1], in_=w2_d[h0:h0 + hn, :])
            nc.sync.dma_start(out=b1_sb[0:hn, hi:hi + 1], in_=b1_d[h0:h0 + hn, :])
            nc.sync.dma_start(out=mb1_sb[0:hn, hi:hi + 1], in_=mb1_d[h0:h0 + hn, :])
            nc.sync.dma_start(out=mb2_sb[0:hn, hi:hi + 1], in_=mb2_d[h0:h0 + hn, :])
            nc.sync.dma_start(out=mw2_sb[0:hn, hi, :], in_=mw2_d[h0:h0 + hn, :])
        make_identity(nc, ident[:])

        # ---- gathers (all 1024-idx calls).  Order: subj pools + main k0
        # first, obj pools + k1, then k2, k3.  queue_map fixes each call's
        # queue to its scheduled DMASW lane (see find_queue_map).
        NHALF = c.T // 2
        nsp = c.gather_split
        npc = NHALF // nsp
        npp = c.NPOOL // 2
        assert npc % 128 == 0 and npp % 128 == 0
        qm = queue_map or {}
        calls = []

        def gather(name, out_ap, in_ap, idxs_ap, n, dflt_q):
            calls.append(name)
            nc.gpsimd.dma_gather(
                out_ap=out_ap, in_ap=in_ap, idxs_ap=idxs_ap, num_idxs=n,
                num_idxs_reg=n, elem_size=c.E, single_packet=False,
                queue_num=qm.get(name, dflt_q))

        lo_win = table[0:32768, :]
        hi_win = table[c.HB2:c.VP, :]

        def main_pair(k, q0, q1):
            i0, s0 = k * (npc // 16), k * (npc // 128)
            gather(f"mlo{k}", emb_tok[:, s0:s0 + npc // 128, :], lo_win,
                   ixl_sb[:, i0:i0 + npc // 16], npc, q0)
            gather(f"mhi{k}", emb_tok[:, c.NS // 2 + s0:c.NS // 2 + s0 + npc // 128, :],
                   hi_win, ixh_sb[:, i0:i0 + npc // 16], npc, q1)

        def pool_pair(j, q0, q1):
            i0, s0 = j * (npp // 16), j * (npp // 128)
            gather(f"plo{j}", pool_lo[:, s0:s0 + npp // 128, :], lo_win,
                   ixpl_sb[:, i0:i0 + npp // 16], npp, q0)
            gather(f"phi{j}", pool_hi[:, s0:s0 + npp // 128, :], hi_win,
                   ixph_sb[:, i0:i0 + npp // 16], npp, q1)

        pool_pair(0, 0, 1)      # subj slots
        main_pair(0, 2, 3)
        pool_pair(1, 0, 1)      # obj slots
        main_pair(1, 2, 3)
        main_pair(2, 0, 1)
        main_pair(3, 2, 3)

        # ---- pools: slotwise lo/hi max, transpose, segmented reduce, clamp.
        # Subject half (subtiles 0..NPS/2) first -- it gates the tanh bias.
        def pool_half(h, ppool):
            st0, st1 = h * (c.NPS // 2), (h + 1) * (c.NPS // 2)
            nc.vector.tensor_tensor(
                out=pcomb[:, st0:st1, :], in0=pool_lo[:, st0:st1, :],
                in1=pool_hi[:, st0:st1, :], op=mybir.AluOpType.max)
            for st in range(st0, st1):
                pp = ppool.tile([c.CW, c.nd, c.PT], BF16, tag="pp")
                for ci in range(c.nd):
                    nc.tensor.transpose(
                        out=pp[:, ci, :],
                        in_=pcomb[:, st, ci * c.CW:(ci + 1) * c.CW],
                        identity=ident[:])
                nc.vector.tensor_reduce(
                    out=pooled[:, :, 2 * st:2 * st + 2],
                    in_=pp[:].rearrange("p c (q s) -> p c q s", s=c.PSLOT),
                    axis=mybir.AxisListType.X, op=mybir.AluOpType.max)
            nc.vector.tensor_scalar_max(
                out=pooled[:, :, 2 * st0:2 * st1],
                in0=pooled[:, :, 2 * st0:2 * st1], scalar1=-NEG_INF)

        def pool_rhs(m, ci, bsl=None):
            dn = c.dch[ci][1]
            if bsl is None:
                bsl = slice(0, c.BC)
            return pooled[0:dn, ci, m * c.BC + bsl.start:m * c.BC + bsl.stop]

        with tc.tile_pool(name="ppool", bufs=1, space="PSUM") as ppool, \
             tc.tile_pool(name="bpool", bufs=1, space="PSUM") as bpool, \
             tc.tile_pool(name="tpool", bufs=1, space="PSUM") as tpool, \
             tc.tile_pool(name="hpool", bufs=3, space="PSUM") as hpool, \
             tc.tile_pool(name="spool", bufs=1, space="PSUM") as spool, \
             tc.tile_pool(name="epool", bufs=2) as epool, \
             tc.tile_pool(name="srpool", bufs=3) as srpool, \
             tc.tile_pool(name="hspool", bufs=2) as hspool:
            pool_half(0, ppool)

            # ---- tanh bias for all batches: w1b^T subj_emb + b1
            for hi, (h0, hn) in enumerate(c.hch):
                pb = bpool.tile([c.HCW, c.BC], F32, tag="pb")
                for ci, (d0, dn) in enumerate(c.dch):
                    nc.tensor.matmul(
                        out=pb[0:hn, :],
                        lhsT=w1b_sb[0:dn, ci, h0:h0 + hn],
                        rhs=pool_rhs(0, ci),
                        start=(ci == 0), stop=(ci == c.nd - 1))
                nc.scalar.activation(
                    out=bias_sb[0:hn, hi, :], in_=pb[0:hn, :],
                    func=mybir.ActivationFunctionType.Identity,
                    bias=b1_sb[0:hn, hi:hi + 1], scale=1.0)

            # ---- grouped main loop: transposes -> dense hid/scores
            GB = min(4, c.BC)
            NG = c.BC // GB
            for g in range(NG):
                embds = []
                for b in range(g * GB, (g + 1) * GB):
                    subs = c.subtiles(b)
                    pt = tpool.tile([c.CW, c.nd, c.L], BF16, tag="pt")
                    for ci in range(c.nd):
                        for si, s in enumerate(subs):
                            nc.tensor.transpose(
                                out=pt[:, ci, si * c.PT:(si + 1) * c.PT],
                                in_=emb_tok[:, s, ci * c.CW:(ci + 1) * c.CW],
                                identity=ident[:])
                    emb_d = epool.tile([c.CW, c.nd, c.L], BF16, tag="embd")
                    nc.scalar.copy(out=emb_d[:, 0, :], in_=pt[:, 0, :])
                    nc.vector.tensor_copy(out=emb_d[:, 1:c.nd, :], in_=pt[:, 1:c.nd, :])
                    embds.append(emb_d)
                for bi, b in enumerate(range(g * GB, (g + 1) * GB)):
                    emb_d = embds[bi]
                    hid = hspool.tile([c.HCW, c.nh, c.L], BF16, tag="hid")
                    for hi, (h0, hn) in enumerate(c.hch):
                        ph = hpool.tile([c.HCW, c.L], F32, tag="ph")
                        for ci, (d0, dn) in enumerate(c.dch):
                            nc.tensor.matmul(
                                out=ph[0:hn, :],
                                lhsT=w1a_sb[0:dn, ci, h0:h0 + hn],
                                rhs=emb_d[0:dn, ci, :],
                                start=(ci == 0), stop=(ci == c.nd - 1))
                        nc.scalar.activation(
                            out=hid[0:hn, hi, :], in_=ph[0:hn, :],
                            func=mybir.ActivationFunctionType.Tanh,
                            bias=bias_sb[0:hn, hi, b:b + 1], scale=1.0)
                    ps = spool.tile([1, c.L], F32, tag="ps")
                    for hi, (h0, hn) in enumerate(c.hch):
                        nc.tensor.matmul(
                            out=ps[:], lhsT=w2_sb[0:hn, hi:hi + 1],
                            rhs=hid[0:hn, hi, :],
                            start=(hi == 0), stop=(hi == c.nh - 1))
                    srow = srpool.tile([1, c.L], F32, tag="srow")
                    nc.vector.tensor_copy(out=srow[:], in_=ps[:])
                    nc.sync.dma_start(out=scores[b:b + 1, :], in_=srow[:])
                if g == 1:
                    pool_half(1, ppool)   # obj pools; only gate the MLP

        # ---- softmax over L for all batches
        nc.vector.tensor_reduce(out=smax[:], in_=scores[:],
                                axis=mybir.AxisListType.X, op=mybir.AluOpType.max)
        nc.vector.tensor_scalar_mul(out=nsmax[:], in0=smax[:], scalar1=-1.0)
        nc.scalar.activation(out=attn[:], in_=scores[:],
                             func=mybir.ActivationFunctionType.Exp,
                             bias=nsmax[:, 0:1], scale=1.0)
        nc.vector.tensor_reduce(out=ssum[:], in_=attn[:],
                                axis=mybir.AxisListType.X, op=mybir.AluOpType.add)
        nc.vector.reciprocal(out=srec[:], in_=ssum[:])
        nc.vector.tensor_scalar_mul(out=attn[:], in0=attn[:], scalar1=srec[:, 0:1])
        nc.vector.tensor_copy(out=attn_bf[:], in_=attn[:])

        # ---- transpose attn to token-major columns [PT, si, b]
        with tc.tile_pool(name="apool", bufs=2, space="PSUM") as apool:
            for si in range(c.NSUB):
                pa = apool.tile([c.PT, c.BC], BF16, tag="pa")
                nc.tensor.transpose(out=pa[:],
                                    in_=attn_bf[:, si * c.PT:(si + 1) * c.PT],
                                    identity=ident[0:c.BC, 0:c.BC])
                nc.vector.tensor_copy(out=attn_t[:, si, :], in_=pa[:])

        # ---- attention-weighted sum  -> sattn [b, D]
        with tc.tile_pool(name="wpool", bufs=4, space="PSUM") as wpool, \
             tc.tile_pool(name="wrpool", bufs=3) as wrpool:
            for b in range(c.BC):
                subs = c.subtiles(b)
                pw = wpool.tile([1, c.D], F32, tag="pw")
                for si, s in enumerate(subs):
                    nc.tensor.matmul(
                        out=pw[:],
                        lhsT=attn_t[:, si, b:b + 1],
                        rhs=emb_tok[:, s, 0:c.D],
                        start=(si == 0), stop=(si == c.NSUB - 1))
                wrow = wrpool.tile([1, c.D], F32, tag="wrow")
                nc.scalar.copy(out=wrow[:], in_=pw[:])
                nc.sync.dma_start(out=sattn[b:b + 1, :], in_=wrow[:])

        # ---- transpose sattn to D-major chunks [dlow, chunk, b]
        nc.vector.tensor_copy(out=sattn_bf[:], in_=sattn[:])
        with tc.tile_pool(name="stpool", bufs=2, space="PSUM") as stpool:
            for ci, (d0, dn) in enumerate(c.dch):
                pst = stpool.tile([c.CW, c.BC], BF16, tag="pst")
                nc.tensor.transpose(out=pst[0:dn, :], in_=sattn_bf[:, d0:d0 + dn],
                                    identity=ident[0:c.BC, 0:c.BC])
                nc.vector.tensor_copy(out=satd[0:dn, ci, :], in_=pst[0:dn, :])

        # ---- output MLP (N=BC is small)
        with tc.tile_pool(name="mpool", bufs=2, space="PSUM") as mpool, \
             tc.tile_pool(name="m2pool", bufs=2, space="PSUM") as m2pool:
            nk = 3 * c.nd
            for hi, (h0, hn) in enumerate(c.hch):
                pm = mpool.tile([c.HCW, c.BC], F32, tag="pm")
                for blk in range(3):
                    for ci, (d0, dn) in enumerate(c.dch):
                        k = blk * c.nd + ci
                        if blk == 0:
                            rhs = satd[0:dn, ci, :]
                        else:
                            rhs = pool_rhs(blk - 1, ci)
                        nc.tensor.matmul(
                            out=pm[0:hn, :],
                            lhsT=mw1_sb[0:dn, k, h0:h0 + hn],
                            rhs=rhs, start=(k == 0), stop=(k == nk - 1))
                nc.scalar.activation(
                    out=o1_sb[0:hn, hi, :], in_=pm[0:hn, :],
                    func=mybir.ActivationFunctionType.Relu,
                    bias=mb1_sb[0:hn, hi:hi + 1], scale=1.0)
            for hi, (h0, hn) in enumerate(c.hch):
                pm2 = m2pool.tile([c.HCW, c.BC], F32, tag="pm2")
                for ki, (k0, kn) in enumerate(c.hch):
                    nc.tensor.matmul(
                        out=pm2[0:hn, :],
                        lhsT=mw2_sb[0:kn, ki, h0:h0 + hn],
                        rhs=o1_sb[0:kn, ki, :],
                        start=(ki == 0), stop=(ki == c.nh - 1))
                nc.scalar.activation(
                    out=out_sb[0:hn, hi, :], in_=pm2[0:hn, :],
                    func=mybir.ActivationFunctionType.Relu,
                    bias=mb2_sb[0:hn, hi:hi + 1], scale=1.0)
            for hi in range(c.nh):
                nc.sync.dma_start(out=out_d[hi], in_=out_sb[:, hi, :])

    nc.finalize()
    nc._gather_call_names = calls
    return nc


def find_queue_map(cfg: Cfg):
    """Assign each gather call the queue implied by its scheduled DMASW
    lane (lane rotates per scheduled SWDGE DMA; a lane's semaphore is
    locked to one queue).  Iterate to a fixed point."""
    import bass_rust
    nsem = bass_rust.NUM_SWDGE_GLOBAL_SEMS
    qm = {}
    for _ in range(4):
        nc = build_nc(cfg, queue_map=qm)
        order = []
        for fn in nc.m.functions:
            for blk in fn.blocks:
                for inst in blk.instructions:
                    if 'Gather' in type(inst).__name__:
                        order.append((inst.name, getattr(inst, 'queue_num', 0)))
        names = {}
        for i, (nm, q) in enumerate(order):
            names[nm] = i
        # map scheduled index -> emission name via creation order
        emit = nc._gather_call_names
        # instructions keep their I-номер names; recover emission index by
        # sorting names (I-<n> increases with emission)
        by_emit = sorted(names.keys(), key=lambda s: int(s.split('-')[1]))
        new_qm = {}
        ok = True
        lane_q = {}
        for nm, q in order:
            emit_idx = by_emit.index(nm)
            lane = names[nm] % nsem
            want = lane_q.setdefault(lane, lane % 4)
            new_qm[emit[emit_idx]] = want
            if q != want:
                ok = False
        qm = new_qm
        if ok:
            return qm
    return qm


# ------------------------------------------------------------------ host


def wrap16(idx, n):
    """int16 index list -> [128, n/16] wrapped + replicated per Q7 core."""
    return np.ascontiguousarray(
        np.tile(np.asarray(idx).astype(np.int16).reshape(n // 16, 16).T, (8, 1)))


def to_bf16(x):
    import ml_dtypes
    return np.asarray(x, dtype=np.float32).astype(ml_dtypes.bfloat16)


def host_prepare(cfg: Cfg, words, subj_pos, obj_pos, emb_table,
                 w1, b1, w2, b2, mw1, mb1, mw2, mb2):
    import ml_dtypes
    c = cfg
    words = np.asarray(words).astype(np.int64)
    subj_pos = np.asarray(subj_pos)
    obj_pos = np.asarray(obj_pos)
    f32 = lambda x: np.ascontiguousarray(np.asarray(x, dtype=np.float32))

    # sentinel rows at physical 0 and VP-1; word w -> row w + 1
    table = np.zeros((c.VP, c.E), ml_dtypes.bfloat16)
    table[1:1 + c.V, :c.D] = to_bf16(emb_table)
    table[0, :] = ml_dtypes.bfloat16(-NEG_INF)
    table[c.VP - 1, :] = ml_dtypes.bfloat16(-NEG_INF)

    w1 = np.asarray(w1, dtype=np.float32)
    w1a, w1b = w1[:c.D], w1[c.D:2 * c.D]
    mw1 = np.asarray(mw1, dtype=np.float32)
    mw1e = np.concatenate([mw1[0:c.D] + mw1[c.D:2 * c.D],
                           mw1[2 * c.D:3 * c.D], mw1[3 * c.D:4 * c.D]], axis=0)
    shared = {
        "table": table,
        "w1a": to_bf16(w1a), "w1b": to_bf16(w1b),
        "b1": f32(b1).reshape(c.H, 1),
        "w2": to_bf16(np.asarray(w2).reshape(c.H, 1)),
        "mw1e": to_bf16(mw1e),
        "mb1": f32(mb1).reshape(c.H, 1),
        "mw2": to_bf16(mw2),
        "mb2": f32(mb2).reshape(c.H, 1),
    }
    HALF = c.L // 2
    in_maps = []
    for core in range(c.NCORES):
        b0 = core * c.BC
        lo_list, hi_list = [], []
        # pool slot arrays, ordered q = m*BC + b
        pl = np.zeros((2 * c.BC, c.PSLOT), np.int64)          # sentinel row 0
        ph = np.full((2 * c.BC, c.PSLOT), 32767, np.int64)    # sentinel VP-1
        for b in range(c.BC):
            w = words[b0 + b]
            order = np.argsort(w, kind="stable")
            ws = w[order] + 1                       # physical rows
            if ws[HALF - 1] >= 32768 or ws[HALF] < c.HB2:
                raise RuntimeError(
                    f"batch {b0 + b}: vocab split infeasible "
                    f"(lo_max={ws[HALF - 1]}, hi_min={ws[HALF]})")
            lo_list.append(ws[:HALF])
            hi_list.append(ws[HALF:] - c.HB2)
            for m, pos in ((0, subj_pos), (1, obj_pos)):
                kept = w[np.asarray(pos[b0 + b]) == 0] + 1    # physical rows
                lo = [int(x) for x in kept[kept < c.HB2]]
                hi = [int(x) for x in kept[kept > 32767]]
                for x in kept[(kept >= c.HB2) & (kept <= 32767)]:
                    (lo if len(lo) < c.PSLOT else hi).append(int(x))
                if len(lo) > c.PSLOT or len(hi) > c.PSLOT:
                    raise RuntimeError(f"pool slot overflow b={b0+b} m={m}: "
                                       f"{len(lo)}/{len(hi)}")
                q = m * c.BC + b
                pl[q, :len(lo)] = lo
                ph[q, :len(hi)] = [x - c.HB2 for x in hi]
        in_maps.append({
            "idx_lo": wrap16(np.concatenate(lo_list), c.T // 2),
            "idx_hi": wrap16(np.concatenate(hi_list), c.T // 2),
            "idx_pl": wrap16(pl.reshape(-1), c.NPOOL),
            "idx_ph": wrap16(ph.reshape(-1), c.NPOOL),
            **shared})
    return in_maps


def assemble_output(cfg: Cfg, results):
    c = cfg
    outs = []
    for core in range(c.NCORES):
        o = results[core]["out"]                      # [nh, HCW, BC]
        outs.append(o.reshape(c.H, c.BC).T)           # [BC, H]
    return np.ascontiguousarray(np.concatenate(outs, axis=0))


_CACHE = {}


def run(inputs, trace=False, **kw):
    from concourse.bass_utils import run_bass_kernel_spmd

    cfg = Cfg()
    in_maps = host_prepare(cfg, **{k: inputs[k] for k in (
        "words", "subj_pos", "obj_pos", "emb_table", "w1", "b1", "w2", "b2",
        "mw1", "mb1", "mw2", "mb2")})
    if "nc" not in _CACHE:
        qm = find_queue_map(cfg)
        _CACHE["nc"] = build_nc(cfg, queue_map=qm)
    nc = _CACHE["nc"]
    res = run_bass_kernel_spmd(nc, in_maps, core_ids=list(range(cfg.NCORES)),
                               trace=trace, **kw)
    return assemble_output(cfg, res.results), res


def kernel(**inputs) -> np.ndarray:
    return run(inputs)[0]
